# revision 3
# baseline (speedup 1.0000x reference)
"""Trainium2 Bass kernel for nn_CrossAttention (B=4, N=2048, E=768, H=8).

Sharding: 8 cores = 4 batches x 2 head-groups (4 heads of 96 dims each).
Each core computes its batch's attention for its 4 heads plus the partial
output projection; the host sums the two head-group partials per batch and
adds bo.

Design (v5, from the 244us v4):
  - v4 kept: 128-col stationaries for FWL; V in exact 97-wide head blocks
    (96 dims + softmax-rowsum ones column via the bias add); scalar engine
    runs ONLY exp; DVE does the O^T copy + reciprocal_approx_fast; rowsum
    reciprocal broadcast via 1-row matmul at tile_position=(96,0); filler
    units drained two per kv-iteration keep the PE busy; qc1 h0+h1 outproj
    partials alias xkv's dead SBUF storage; PSUM = exactly 8 banks.
  - v5 head: input DMAs are split so the first projection's operands land
    ~7us after boot (wk/wq go in block-major host layout so 128-col weight
    chunks stay >=512B/descriptor; x tensors stream in 512-token waves).
    Warmup drops 62 -> 6 dummy matmuls; real work starts at PE boot.
  - v5 schedule: outproj fillers unlock one block earlier (mid-blk4 for
    qc0, mid-blk6 for h0+h1 partials), drained inside the kv loops.
  - v5 tail: the last attention block runs as two 512-token half-blocks;
    the first half's h2+h3 outproj units overlap the second half's
    attention, so only ~4 units + one store remain after the last PV.
  - rings: scalar carries one early wave then is free for exp; vector one
    early wave then free for DVE; gpsimd carries weights then repacks;
    sync carries the rest; output stores rotate sync/gpsimd (scalar only
    for tail stores once exp is done).
"""

import os
import sys
import types
from collections import deque

import numpy as np

# ---------------------------------------------------------------------------
# NTFF profile hook (the agent image's antenv lacks axon_hooks; degrade OK)
# ---------------------------------------------------------------------------
def _install_ntff_hook():
    if "antenv.axon_hooks" in sys.modules:
        return
    try:
        hooks = types.ModuleType("antenv.axon_hooks")
        hooks._hook = None
        hooks.set_axon_ntff_profile_hook = lambda h: setattr(hooks, "_hook", h)
        hooks.get_axon_ntff_profile_hook = lambda: hooks._hook
        sys.modules["antenv.axon_hooks"] = hooks
        import antenv

        antenv.axon_hooks = hooks
        from trn_agent_boot.trn_boot import _ntff_profile_via_ctypes

        so = "/opt/axon/libaxon_pjrt.so"
        if os.path.exists(so):
            hooks.set_axon_ntff_profile_hook(_ntff_profile_via_ctypes(so))
    except Exception:
        pass


_install_ntff_hook()

import concourse.bacc as bacc
import concourse.tile as tile
import concourse.mybir as mybir
from concourse import bass_utils
from concourse.alu_op_type import AluOpType

F32 = mybir.dt.float32
F32R = mybir.dt.float32r
BF16 = mybir.dt.bfloat16

B = 4
NQ = 2048
NKV = 2048
E = 768
H_LOCAL = 4  # heads per core
HD = 96  # head dim
HP = 128  # padded head dim
D = H_LOCAL * HD  # 384 local proj dim
VW = HD + 1  # 97: per-head V block (96 dims + rowsum ones column)
DV = H_LOCAL * VW  # 388
ET = E // 128  # 6 contraction tiles
KV_T = NKV // 128  # 16 kv tiles
QT_T = NQ // 128  # 16 q tiles
INV_SQRT_E = 1.0 / float(np.sqrt(np.float32(E)))


def build_nc():
    nc = bacc.Bacc("TRN2", target_bir_lowering=False, debug=False)

    xq_t = nc.dram_tensor("xq_t", [E, NQ], BF16, kind="ExternalInput")
    xkv_t = nc.dram_tensor("xkv_t", [E, NKV], BF16, kind="ExternalInput")
    # wq/wk in block-major layout: row (t*128+p), col (e*128+n) holds
    # W^T[e*128+p, t*128+n] so a 128-col chunk is one 1.5KB row/descriptor.
    wq_t = nc.dram_tensor("wq_t", [D, E], BF16, kind="ExternalInput")
    wk_t = nc.dram_tensor("wk_t", [D, E], BF16, kind="ExternalInput")
    wv_t = nc.dram_tensor("wv_t", [E, DV], BF16, kind="ExternalInput")
    wo_t = nc.dram_tensor("wo_t", [D, E], BF16, kind="ExternalInput")
    consts_t = nc.dram_tensor("consts_t", [128, 6 + DV], F32, kind="ExternalInput")
    out = nc.dram_tensor("out", [NQ, E], F32, kind="ExternalOutput")

    with tile.TileContext(nc) as tc:
        with (
            nc.allow_low_precision(reason="bf16 matmuls and f32r broadcast"),
            tc.tile_pool(name="persist", bufs=1) as persist,
            tc.tile_pool(name="psum", bufs=1, space="PSUM") as pp,
            tc.tile_pool(name="sb", bufs=1) as sb,
        ):
            # ---------------- persistent SBUF tensors ----------------
            KT = persist.tile([HD, H_LOCAL, NKV], BF16)  # K^T per head
            QT = persist.tile([HD, H_LOCAL, NQ], BF16)  # Q^T per head
            V = persist.tile([128, KV_T, DV], BF16)
            attn = persist.tile([HD, H_LOCAL, NQ], BF16)  # normalized attn^T
            wo_sb = persist.tile([HD, H_LOCAL, E], BF16)
            wq_sb = persist.tile([128, ET, D], BF16)
            wk_sb = persist.tile([128, ET, D], BF16)
            wv_sb = persist.tile([128, ET, DV], BF16)
            K3 = persist.tile([128, 3, NKV], BF16)  # packed K^T staging
            Q3 = persist.tile([128, 3, NQ], BF16)  # packed Q^T staging
            consts_sb = persist.tile([128, 6 + DV], F32)
            bk_sb = consts_sb[:, 0:3]
            bq_sb = consts_sb[:, 3:6]
            bv_sb = consts_sb[:, 6 : 6 + DV]
            ones_sb = persist.tile([HD + 1, HP], BF16)  # row 96 = [1]*96+[0]*32
            xkv_sb = persist.tile([128, ET, NKV], BF16)
            # qc1 outproj h0+h1 partials alias xkv's storage (dead by then):
            ob01 = xkv_sb[:].bitcast(F32).rearrange("p t n -> p (t n)").rearrange(
                "p (a b) -> p a b", a=8
            )
            xq_sb = persist.tile([128, ET, NQ], BF16)

            # ---------------- input DMAs ----------------
            # One shared ~360GB/s DMA pool; order = priority. scalar gets a
            # single early wave (free for exp by ~10us), vector one wave
            # (free for DVE by ~9us), gpsimd the weights, sync the rest.
            def wblk(dst_sb, src_t, t3):
                return (
                    dst_sb[:, :, t3 * 128 : (t3 + 1) * 128],
                    src_t[t3 * 128 : (t3 + 1) * 128, :].rearrange(
                        "p (e n) -> p e n", n=128
                    ),
                )

            def xwave(dst_sb, src_t, c):
                return (
                    dst_sb[:, :, c * 512 : (c + 1) * 512],
                    src_t[:, c * 512 : (c + 1) * 512].rearrange(
                        "(t p) n -> p t n", p=128
                    ),
                )

            nc.gpsimd.dma_start(*wblk(wk_sb, wk_t, 0))
            nc.sync.dma_start(consts_sb[:], consts_t[:])
            nc.sync.dma_start(*xwave(xkv_sb, xkv_t, 0))
            nc.scalar.dma_start(*xwave(xq_sb, xq_t, 0))
            nc.gpsimd.dma_start(*wblk(wq_sb, wq_t, 0))
            nc.gpsimd.dma_start(
                wv_sb[:], wv_t[:].rearrange("(t p) n -> p t n", p=128)
            )
            nc.scalar.dma_start(*xwave(xq_sb, xq_t, 1))
            nc.sync.dma_start(*xwave(xkv_sb, xkv_t, 1))
            nc.sync.dma_start(*xwave(xkv_sb, xkv_t, 2))
            nc.gpsimd.dma_start(*wblk(wk_sb, wk_t, 1))
            nc.gpsimd.dma_start(*wblk(wk_sb, wk_t, 2))
            nc.sync.dma_start(*xwave(xkv_sb, xkv_t, 3))
            nc.gpsimd.dma_start(*wblk(wq_sb, wq_t, 1))
            nc.gpsimd.dma_start(*wblk(wq_sb, wq_t, 2))
            nc.sync.dma_start(
                wo_sb[:], wo_t[:].rearrange("(h p) n -> p h n", p=HD)
            )
            nc.sync.dma_start(*xwave(xq_sb, xq_t, 2))
            nc.sync.dma_start(*xwave(xq_sb, xq_t, 3))
            nc.vector.memset(ones_sb[HD : HD + 1, 0:HD], 1.0)
            nc.vector.memset(ones_sb[HD : HD + 1, HD:HP], 0.0)

            # ---------------- PE warm-up ----------------
            warm = persist.tile([128, 512], BF16)
            nc.vector.memset(warm[:], 0.0)

            # ---------------- psum slot helpers ----------------
            # tags: s 2x[128,1024], po 1x[128,1024], x 2x[128,512]: 8 banks
            def psum_s():
                return pp.tile([128, 1024], F32, tag="s", bufs=2, name="ps_s")

            def psum_po():
                return pp.tile([128, 1024], F32, tag="po", bufs=1, name="ps_po")

            def psum_x():
                return pp.tile([128, 512], F32, tag="x", bufs=2, name="ps_x")

            # ---------------- projection / outproj unit emitters ----------
            def kp_unit(w_sb, b3, dst3, x_sb, t3, c):
                ps = psum_x()
                for e in range(ET):
                    nc.tensor.matmul(
                        ps[:],
                        w_sb[:, e, t3 * 128 : (t3 + 1) * 128],
                        x_sb[:, e, c * 512 : (c + 1) * 512],
                        start=(e == 0),
                        stop=(e == ET - 1),
                    )
                nc.vector.tensor_scalar_add(
                    out=dst3[:, t3, c * 512 : (c + 1) * 512],
                    in0=ps[:],
                    scalar1=b3[:, t3 : t3 + 1],
                )

            def repack_unit(src3, dst, h, c):
                ring = nc.gpsimd
                cs = slice(c * 512, (c + 1) * 512)
                lo = h * HD
                t_lo, r_lo = divmod(lo, 128)
                n0 = min(128 - r_lo, HD)
                ring.dma_start(
                    dst[0:n0, h, cs], src3[r_lo : r_lo + n0, t_lo, cs]
                )
                if n0 < HD:
                    ring.dma_start(
                        dst[n0:HD, h, cs],
                        src3[0 : HD - n0, t_lo + 1, cs],
                    )

            def v_unit(t):
                ps = psum_x()
                for e in range(ET):
                    nc.tensor.matmul(
                        ps[:, 0:DV],
                        xkv_sb[:, e, t * 128 : (t + 1) * 128],
                        wv_sb[:, e, :],
                        start=(e == 0),
                        stop=(e == ET - 1),
                    )
                nc.vector.tensor_tensor(
                    out=V[:, t, :],
                    in0=ps[:, 0:DV],
                    in1=bv_sb[:],
                    op=AluOpType.add,
                )

            ob_i = [0]

            def op_unit(qt):
                # out[qt] = attn^T_qt.T @ Wo^T  (accumulate 4 heads), qc0
                fa = psum_x()
                fb = psum_x()
                for h in range(H_LOCAL):
                    nc.tensor.matmul(
                        fa[:],
                        attn[:, h, qt * 128 : (qt + 1) * 128],
                        wo_sb[:, h, 0:512],
                        start=(h == 0),
                        stop=(h == H_LOCAL - 1),
                    )
                for h in range(H_LOCAL):
                    nc.tensor.matmul(
                        fb[:, 0:256],
                        attn[:, h, qt * 128 : (qt + 1) * 128],
                        wo_sb[:, h, 512:768],
                        start=(h == 0),
                        stop=(h == H_LOCAL - 1),
                    )
                ob = sb.tile([128, E], F32, tag="ob", bufs=2, name="ob")
                nc.vector.tensor_copy(ob[:, 0:512], fa[:])
                nc.vector.tensor_copy(ob[:, 512:768], fb[:, 0:256])
                ring = (nc.sync, nc.gpsimd)[ob_i[0] % 2]
                ob_i[0] += 1
                ring.dma_start(out[qt * 128 : (qt + 1) * 128, :], ob[:])

            def op_h01_unit(qt):
                fa = psum_x()
                fb = psum_x()
                for h in range(2):
                    nc.tensor.matmul(
                        fa[:],
                        attn[:, h, qt * 128 : (qt + 1) * 128],
                        wo_sb[:, h, 0:512],
                        start=(h == 0),
                        stop=(h == 1),
                    )
                    nc.tensor.matmul(
                        fb[:, 0:256],
                        attn[:, h, qt * 128 : (qt + 1) * 128],
                        wo_sb[:, h, 512:768],
                        start=(h == 0),
                        stop=(h == 1),
                    )
                nc.vector.tensor_copy(ob01[:, qt - 8, 0:512], fa[:])
                nc.vector.tensor_copy(ob01[:, qt - 8, 512:768], fb[:, 0:256])

            def op_h23_unit(qt, fa, fb, ring_a, ring_b):
                for h in range(2, 4):
                    nc.tensor.matmul(
                        fa,
                        attn[:, h, qt * 128 : (qt + 1) * 128],
                        wo_sb[:, h, 0:512],
                        start=(h == 2),
                        stop=(h == 3),
                    )
                    nc.tensor.matmul(
                        fb,
                        attn[:, h, qt * 128 : (qt + 1) * 128],
                        wo_sb[:, h, 512:768],
                        start=(h == 2),
                        stop=(h == 3),
                    )
                ob = sb.tile([128, E], F32, tag="ob", bufs=2, name="ob")
                nc.vector.tensor_tensor(
                    out=ob[:, 0:512], in0=fa, in1=ob01[:, qt - 8, 0:512],
                    op=AluOpType.add,
                )
                ring_a.dma_start(
                    out[qt * 128 : (qt + 1) * 128, 0:512], ob[:, 0:512]
                )
                nc.vector.tensor_tensor(
                    out=ob[:, 512:768], in0=fb, in1=ob01[:, qt - 8, 512:768],
                    op=AluOpType.add,
                )
                ring_b.dma_start(
                    out[qt * 128 : (qt + 1) * 128, 512:768], ob[:, 512:768]
                )

            def op_h23_x(qt, ring_a, ring_b):
                fa = psum_x()
                fb = psum_x()
                op_h23_unit(qt, fa[:], fb[:, 0:256], ring_a, ring_b)

            # ---------------- filler machinery ----------------
            fillerA = deque()  # (deadline_block, fn)
            fillerB = deque()  # qc0 outproj units (unlock mid-blk4)
            fillerC = deque()  # qc1 h0+h1 partials (unlock mid-blk6)
            fillerD = deque()  # qc1 h2+h3 qt8-11 (unlock mid-blk7B)
            b_unlocked = [False]
            c_unlocked = [False]
            d_unlocked = [False]

            def drain_one():
                if fillerA:
                    fillerA.popleft()[1]()
                elif b_unlocked[0] and fillerB:
                    fillerB.popleft()()
                elif c_unlocked[0] and fillerC:
                    fillerC.popleft()()
                elif d_unlocked[0] and fillerD:
                    fillerD.popleft()()

            def drain_deadline(blk):
                while fillerA and fillerA[0][0] <= blk:
                    fillerA.popleft()[1]()

            # ---------------- attention block ----------------
            def norm_rest_for(o_sb, rs, h, tok0, nch):
                def norm_rest():
                    for n in range(nch):
                        bcT = psum_x()
                        nc.tensor.matmul(
                            bcT[:],
                            ones_sb[HD : HD + 1, :],
                            rs[HD : HD + 1, n * 512 : (n + 1) * 512],
                            start=True,
                            stop=True,
                            tile_position=(96, 0),
                        )
                        nc.vector.tensor_tensor(
                            out=attn[
                                :,
                                h,
                                (tok0 + n) * 512 : (tok0 + n + 1) * 512,
                            ],
                            in0=o_sb[0:HD, n * 512 : (n + 1) * 512],
                            in1=bcT[0:HD, :],
                            op=AluOpType.mult,
                        )

                return norm_rest

            def attn_span(h, tok0, nch, prev_norm, blk, unlock=None):
                # attention for head h over q-tokens [tok0*512, (tok0+nch)*512)
                drain_deadline(blk)
                po = psum_po()
                pc0 = (tok0 % 2) * 512
                poh = po[:, pc0 : pc0 + nch * 512]
                if h == 0:
                    KTh = K3[0:HD, 0, :]
                    QTh = Q3[0:HD, 0, :]
                else:
                    KTh = KT[:, h, :]
                    QTh = QT[:, h, :]
                w = nch * 512
                p_prev = None
                for kv in range(KV_T):
                    s = psum_s()
                    for n in range(nch):
                        nc.tensor.matmul(
                            s[:, n * 512 : (n + 1) * 512],
                            KTh[:, kv * 128 : (kv + 1) * 128],
                            QTh[:, (tok0 + n) * 512 : (tok0 + n + 1) * 512],
                            start=True,
                            stop=True,
                        )
                    p = sb.tile([128, 1024], BF16, tag="p", bufs=3, name="p")
                    nc.scalar.activation(
                        p[:, 0:w], s[:, 0:w], mybir.ActivationFunctionType.Exp,
                        scale=INV_SQRT_E,
                    )
                    if kv == 7:
                        if prev_norm is not None:
                            prev_norm()
                        if unlock is not None:
                            unlock()
                    drain_one()
                    drain_one()
                    if p_prev is not None:
                        for n in range(nch):
                            nc.tensor.matmul(
                                poh[0:VW, n * 512 : (n + 1) * 512],
                                V[:, kv - 1, h * VW : (h + 1) * VW],
                                p_prev[:, n * 512 : (n + 1) * 512],
                                start=(kv == 1),
                                stop=False,
                            )
                    p_prev = p
                for n in range(nch):
                    nc.tensor.matmul(
                        poh[0:VW, n * 512 : (n + 1) * 512],
                        V[:, KV_T - 1, h * VW : (h + 1) * VW],
                        p_prev[:, n * 512 : (n + 1) * 512],
                        start=False,
                        stop=True,
                    )
                o_sb = sb.tile([HD + 1, 1024], F32, tag="osb", bufs=2, name="o_sb")
                nc.vector.tensor_copy(o_sb[:, 0:w], poh[0 : HD + 1, :])
                rs = sb.tile([HD + 1, 1024], BF16, tag="rs", bufs=2, name="rs")
                rf = sb.tile([HD + 1, 1024], F32, tag="rf", bufs=2, name="rf")
                nc.vector.reciprocal_approx_fast(
                    out=rf[:, 0:w], in_=o_sb[:, 0:w]
                )
                nc.vector.tensor_copy(rs[HD : HD + 1, 0:w], rf[HD : HD + 1, 0:w])
                return norm_rest_for(o_sb, rs, h, tok0, nch)

            # ---------------- preamble ----------------
            for _ in range(6):
                wps = psum_x()
                nc.tensor.matmul(
                    wps[:], warm[:, 0:128], warm[:], start=True, stop=True
                )
            kp_unit(wk_sb, bk_sb, K3, xkv_sb, 0, 0)
            kp_unit(wq_sb, bq_sb, Q3, xq_sb, 0, 0)
            v_unit(0)
            v_unit(1)
            kp_unit(wk_sb, bk_sb, K3, xkv_sb, 0, 1)
            kp_unit(wq_sb, bq_sb, Q3, xq_sb, 0, 1)

            # ---------------- filler queues ----------------
            def K_unit(t3, c):
                return lambda: kp_unit(wk_sb, bk_sb, K3, xkv_sb, t3, c)

            def Q_unit(t3, c):
                return lambda: kp_unit(wq_sb, bq_sb, Q3, xq_sb, t3, c)

            def RK(h, c):
                return lambda: repack_unit(K3, KT, h, c)

            def RQ(h, c):
                return lambda: repack_unit(Q3, QT, h, c)

            def VU(t):
                return lambda: v_unit(t)

            blk0 = [VU(2), VU(3), VU(4), VU(5), VU(6), VU(7),
                    K_unit(0, 2), VU(8), VU(9), VU(10), VU(11),
                    K_unit(0, 3), VU(12), VU(13), VU(14), VU(15)]
            for c in range(4):
                blk0 += [K_unit(1, c), RK(1, c)]
            for c in range(2):
                blk0 += [Q_unit(1, c), RQ(1, c)]
            blk1 = []
            for c in range(4):
                blk1 += [K_unit(2, c), RK(2, c)]
            for c in range(2):
                blk1 += [Q_unit(2, c), RQ(2, c)]
            blk2 = []
            for c in range(4):
                blk2 += [RK(3, c)]
            for c in range(2):
                blk2 += [RQ(3, c)]
            blk3 = []
            for c in range(2, 4):
                blk3 += [Q_unit(0, c), Q_unit(1, c), Q_unit(2, c),
                         RQ(1, c), RQ(2, c), RQ(3, c)]
            fillerA = deque(
                [(1, f) for f in blk0]
                + [(2, f) for f in blk1]
                + [(3, f) for f in blk2]
                + [(4, f) for f in blk3]
            )
            for qt in range(8):  # outproj for qc0
                fillerB.append(lambda qt=qt: op_unit(qt))
            for qt in range(8, QT_T):  # qc1 h0+h1 partials
                fillerC.append(lambda qt=qt: op_h01_unit(qt))
            for qt in range(8, 12):  # qc1 h2+h3 for the 7A half
                fillerD.append(
                    lambda qt=qt: op_h23_x(qt, nc.sync, nc.gpsimd)
                )

            def unlock_b():
                b_unlocked[0] = True

            def unlock_c():
                c_unlocked[0] = True

            def unlock_d():
                d_unlocked[0] = True

            # ---------------- main loop ----------------
            # spans: (head, first 512-chunk, n chunks, unlock)
            spans = [
                (0, 0, 2, None), (1, 0, 2, None),
                (2, 0, 2, None), (3, 0, 2, None),
                (0, 2, 2, unlock_b), (1, 2, 2, None),
                (2, 2, 2, unlock_c), (3, 2, 1, None),
                (3, 3, 1, unlock_d),
            ]
            prev_norm = None
            for blk, (h, tok0, nch, unlock) in enumerate(spans):
                prev_norm = attn_span(
                    h, tok0, nch, prev_norm, min(blk, 7), unlock
                )
            # tail: last norm, remaining fillers, outproj qc1 h2+h3 tail
            prev_norm()
            while fillerA:
                fillerA.popleft()[1]()
            b_unlocked[0] = True
            while fillerB:
                fillerB.popleft()()
            c_unlocked[0] = True
            while fillerC:
                fillerC.popleft()()
            d_unlocked[0] = True
            while fillerD:
                fillerD.popleft()()
            for i, qt in enumerate(range(12, QT_T)):
                m = i % 3
                if m == 0:
                    fa = psum_x()
                    fb = psum_x()
                    op_h23_unit(qt, fa[:], fb[:, 0:256], nc.sync, nc.scalar)
                elif m == 1:
                    t = psum_s()
                    op_h23_unit(qt, t[:, 0:512], t[:, 512:768], nc.sync, nc.scalar)
                else:
                    t = psum_po()
                    op_h23_unit(qt, t[:, 0:512], t[:, 512:768], nc.sync, nc.scalar)

    nc.compile()
    return nc


_NC_CACHE = None


def _prep_inputs(x_query, x_kv, Wq, bq, Wk, bk, Wv, bv, Wo, bo):
    import ml_dtypes

    bf16 = ml_dtypes.bfloat16

    def pad_w(W_sl):
        # [384, 768] -> [768, 4, 97] with a zero ones-column, -> [768, 388]
        wp = np.zeros((E, H_LOCAL, VW), dtype=np.float32)
        wt = W_sl.T  # [768, 384]
        for h in range(H_LOCAL):
            wp[:, h, 0:HD] = wt[:, h * HD : (h + 1) * HD]
        return np.ascontiguousarray(wp.reshape(E, DV)).astype(bf16)

    def pack_w_blocks(W_sl):
        # [384, 768] -> block-major [384, 768]: row (t*128+p), col (e*128+n)
        # = W_sl.T[e*128+p, t*128+n]
        X = W_sl.T.reshape(ET, 128, 3, 128)  # [e, p, t, n]
        X = X.transpose(2, 1, 0, 3).reshape(D, E)  # [t, p, e, n]
        return np.ascontiguousarray(X).astype(bf16)

    in_maps = []
    for c in range(8):
        b, g = divmod(c, 2)
        sl = slice(g * D, (g + 1) * D)
        bve = np.zeros((DV,), dtype=np.float32)
        for h in range(H_LOCAL):
            bve[h * VW : h * VW + HD] = bv[sl][h * HD : (h + 1) * HD]
            bve[h * VW + HD] = 1.0
        consts = np.zeros((128, 6 + DV), dtype=np.float32)
        consts[:, 0:3] = bk[sl].reshape(3, 128).T
        consts[:, 3:6] = bq[sl].reshape(3, 128).T
        consts[:, 6 : 6 + DV] = bve[None, :]
        in_maps.append(
            {
                "xq_t": np.ascontiguousarray(x_query[b].T).astype(bf16),
                "xkv_t": np.ascontiguousarray(x_kv[b].T).astype(bf16),
                "wq_t": pack_w_blocks(Wq[sl, :]),
                "wk_t": pack_w_blocks(Wk[sl, :]),
                "wv_t": pad_w(Wv[sl, :]),
                "wo_t": np.ascontiguousarray(Wo[:, sl].T).astype(bf16),
                "consts_t": consts,
            }
        )
    return in_maps


def kernel(x_query, x_kv, Wq, bq, Wk, bk, Wv, bv, Wo, bo):
    global _NC_CACHE
    x_query = np.asarray(x_query, dtype=np.float32)
    x_kv = np.asarray(x_kv, dtype=np.float32)
    Wq = np.asarray(Wq, dtype=np.float32)
    Wk = np.asarray(Wk, dtype=np.float32)
    Wv = np.asarray(Wv, dtype=np.float32)
    Wo = np.asarray(Wo, dtype=np.float32)
    bq = np.asarray(bq, dtype=np.float32)
    bk = np.asarray(bk, dtype=np.float32)
    bv = np.asarray(bv, dtype=np.float32)
    bo = np.asarray(bo, dtype=np.float32)

    if _NC_CACHE is None:
        _NC_CACHE = build_nc()
    nc = _NC_CACHE

    in_maps = _prep_inputs(x_query, x_kv, Wq, bq, Wk, bk, Wv, bv, Wo, bo)

    trace = bool(int(os.environ.get("KERNEL_TRACE", "0")))
    res = bass_utils.run_bass_kernel_spmd(
        nc, in_maps, core_ids=list(range(8)), trace=trace
    )
    if trace:
        kernel.last_exec_time_ns = res.exec_time_ns
        kernel.last_results = res

    out = np.empty((B, NQ, E), dtype=np.float32)
    for b in range(B):
        out[b] = res.results[2 * b]["out"] + res.results[2 * b + 1]["out"] + bo
    return out
